# revision 28
# baseline (speedup 1.0000x reference)
"""Multi-head causal attention (B=2, T=2048, D=1024, H=16, Hd=64) on 8 trn2 cores.

Sharding: batch x head-group. Core c handles batch c//4 and heads
(c%4)*4 .. (c%4)*4+3 (data + tensor/head parallel). Each core computes
Q/K/V projections for its 4 heads, causal attention, and a partial
output projection (row-slice of Wo); the host sums the 4 bf16 partials
per batch and adds bo_eff = bo + bv @ Wo (bv commutes through softmax,
so it is folded into the host-side output bias and dropped on-device).

Device layout notes:
- Host passes x^T (q/k/v transposed to [D, T]) in bf16 so every matmul
  contraction has its operand partition-major; no on-chip transposes.
  Projection weights are host-packed to the on-chip [p, k, n] layout so
  their DMAs move 4KB contiguous rows (a strided load runs ~55GB/s and
  would gate the first projection by ~10us).
- Scores are computed transposed (S^T[t2, t1] = K^T.T @ Q^T) so softmax
  sums land on the PE via an appended ones-column in V (row 64 of the
  O^T psum accumulates the denominators for free).
- No max-subtraction in softmax: scaled scores are bounded (|S|/8 < 9
  for N(0,1)-scale inputs; exp stays far from fp32 overflow).
- Normalization: DVE copy of the psum denominator row (the only
  cross-partition hop hardware DVE supports) -> reciprocal -> gpsimd
  partition_broadcast -> DVE multiply into O^T (bf16).

Scheduling notes (HAM clock gate: PE runs 1.2 GHz until ~3.4us of
sustained high-utilization activity, 2.4 GHz after; re-throttles after
an idle window; K=1-contraction matmuls do NOT register as activity):
- The 12.6MB x load is HBM-bandwidth-bound (~36-45us at the per-core
  roofline). x streams as column-halves, earliest-needed-first, over
  the gpsimd/sync/scalar trigger queues; K=128 warmup matmuls (memset
  operands, no DMA deps) bridge until the first slices land.
- Q/K projections are split into 4096-cycle units and dripped into the
  attention streams as PE filler, paced so each unit trails its data's
  arrival; attention chunk 0 runs S-first (all S pairs before any PV)
  so nothing waits on the later x_v stream.
- Output projection runs as [128,512] half-units dripped one chunk
  late, writing bf16 partials (host sums in f32); warm-keeper matmuls
  cover the last norm's DVE/gpsimd latency so the tail stays at 2.4GHz.
"""

import os
import sys

for _p in ("/root/.axon_site/_ro/trn_rl_repo", "/opt/trn_rl_repo"):
    if _p not in sys.path and os.path.isdir(_p):
        sys.path.append(_p)

import numpy as np
import ml_dtypes

B, T, D = 2, 2048, 1024
H, HD = 16, 64
HPC = 4                # heads per core
DH = HPC * HD          # 256 head-dim cols per core
KC = D // 128          # 8 contraction chunks
NT4 = T // 512         # 4 t1-chunks
NB = T // 128          # 16 t2-blocks
N_CORES = 8
N_WARMUP = 8

_BF16 = ml_dtypes.bfloat16
_cache = {}


def _build():
    import concourse.bass as bass
    import concourse.tile as tile
    from concourse import bacc, mybir

    f32 = mybir.dt.float32
    bf16 = mybir.dt.bfloat16
    Exp = mybir.ActivationFunctionType.Exp
    Identity = mybir.ActivationFunctionType.Identity

    nc = bacc.Bacc(target_bir_lowering=False)

    xqt_d = nc.declare_dram_parameter("xqt", [D, T], bf16, isOutput=False)
    xkt_d = nc.declare_dram_parameter("xkt", [D, T], bf16, isOutput=False)
    xvt_d = nc.declare_dram_parameter("xvt", [D, T], bf16, isOutput=False)
    wq_d = nc.declare_dram_parameter("wq", [128, KC * DH], bf16, isOutput=False)
    wk_d = nc.declare_dram_parameter("wk", [128, KC * DH], bf16, isOutput=False)
    wv_d = nc.declare_dram_parameter("wv", [128, KC * DH], bf16, isOutput=False)
    wo_d = nc.declare_dram_parameter("wo", [128, 2 * D], bf16, isOutput=False)
    bqk_d = nc.declare_dram_parameter("bqk", [128, 4], f32, isOutput=False)
    tri_d = nc.declare_dram_parameter("tri", [128, 128], bf16, isOutput=False)
    out_d = nc.declare_dram_parameter("out", [T, D], bf16, isOutput=True)

    with tile.TileContext(nc) as tc:
        with tc.tile_pool(name="res", bufs=1) as res, \
             tc.tile_pool(name="ptp", bufs=12) as ptp, \
             tc.tile_pool(name="outp", bufs=3) as outp, \
             tc.tile_pool(name="recp", bufs=2) as recp, \
             tc.tile_pool(name="bcp", bufs=2) as bcp, \
             tc.tile_pool(name="ps_a", bufs=2, space="PSUM") as ps_a, \
             tc.tile_pool(name="ps_b", bufs=2, space="PSUM") as ps_b, \
             tc.tile_pool(name="ps_o", bufs=1, space="PSUM") as ps_o:

            # ---- warmup operands: no DMA dependency ----
            # K=128 so HAM's activity monitor actually sees the PE as busy
            # (a K=1 warmup exercises 1/128 of the array and reads as idle)
            warm_l = res.tile([128, 128], bf16, name="warm_l")
            warm_r = res.tile([128, 512], bf16, name="warm_r")
            nc.vector.memset(warm_l[:], 0.01)
            nc.vector.memset(warm_r[:], 0.01)

            # ---- persistent tiles ----
            wq_sb = res.tile([128, KC, DH], bf16, name="wq")
            wk_sb = res.tile([128, KC, DH], bf16, name="wk")
            wv_sb = res.tile([128, KC, DH], bf16, name="wv")
            wo_sb = res.tile([128, 2, D], bf16, name="wo")
            bqk_sb = res.tile([128, 4], f32, name="bqk")
            bq_sb = bqk_sb[:, 0:2]
            bk_sb = bqk_sb[:, 2:4]
            tri_sb = res.tile([128, 128], bf16, name="tri")
            xq = [res.tile([128, T], bf16, name=f"xq{k}") for k in range(KC)]
            xk = [res.tile([128, T], bf16, name=f"xk{k}") for k in range(KC)]
            xv = [res.tile([128, T], bf16, name=f"xv{k}") for k in range(KC)]
            qt_sb = [res.tile([128, T], bf16, name=f"qt{i}") for i in range(2)]
            kt_sb = [res.tile([128, T], bf16, name=f"kt{i}") for i in range(2)]
            ont_sb = [res.tile([128, T], bf16, name=f"ont{i}") for i in range(2)]
            vaug_sb = res.tile([128, NB, HPC * (HD + 1)], bf16, name="vaug")
            # ones columns for the denominator trick
            nc.vector.memset(
                vaug_sb[:].rearrange("p b (h x) -> p b h x", h=HPC)[:, :, :, HD : HD + 1],
                1.0,
            )

            # ---- DMA triggers: column-sliced, earliest-needed-first ----
            # The 12.6MB input load is HBM-bandwidth-bound (~36-45us at the
            # per-core roofline), so x streams in t-column slices ordered by
            # when the schedule consumes them, interleaved round-robin over
            # all three trigger queues (gpsimd/sync/scalar). Weights are
            # host-packed to the on-chip layout so their DMAs move 4KB
            # contiguous rows (a strided [1024,256] load runs ~55GB/s and
            # would gate the first projection by ~10us).
            nc.gpsimd.dma_start(out=wq_sb[:].rearrange("p k n -> p (k n)"), in_=wq_d[:])
            nc.sync.dma_start(out=wk_sb[:].rearrange("p k n -> p (k n)"), in_=wk_d[:])
            # x staging, earliest-needed-first, balanced across queues:
            #   t01 halves of q/k alternate over gpsimd/sync; xv spreads
            #   over all three queues (a scalar-only xv stream is ~26us
            #   serial and stalls chunk-0/1 PV); the second halves load as
            #   t2-then-t3 quarters so chunk 2's projection data lands
            #   ~10us earlier than a monolithic half would.
            def xpart(eng, xd, xt, k, c0, c1):
                eng.dma_start(
                    out=xt[k][:, c0:c1],
                    in_=xd[k * 128 : (k + 1) * 128, c0:c1],
                )
            ge, se, sc = nc.gpsimd, nc.sync, nc.scalar
            for k in range(KC):
                xpart(ge if k % 2 == 0 else se, xqt_d, xq, k, 0, 1024)
                xpart(se if k % 2 == 0 else ge, xkt_d, xk, k, 0, 1024)
            sc.dma_start(out=bqk_sb[:], in_=bqk_d[:])
            sc.dma_start(out=tri_sb[:], in_=tri_d[:])
            sc.dma_start(out=wv_sb[:].rearrange("p k n -> p (k n)"), in_=wv_d[:])
            for k in range(4):
                xpart(sc, xvt_d, xv, k, 0, 1024)
            xpart(ge, xvt_d, xv, 4, 0, 1024)
            xpart(se, xvt_d, xv, 5, 0, 1024)
            xpart(ge, xvt_d, xv, 6, 0, 1024)
            xpart(se, xvt_d, xv, 7, 0, 1024)
            sc.dma_start(out=wo_sb[:].rearrange("p c n -> p (c n)"), in_=wo_d[:])
            for t4 in (2, 3):
                for k in range(KC):
                    xpart(ge if k % 2 == 0 else se, xqt_d, xq, k, t4 * 512, (t4 + 1) * 512)
                    xpart(se if k % 2 == 0 else ge, xkt_d, xk, k, t4 * 512, (t4 + 1) * 512)
            for k in range(4):
                xpart(sc, xvt_d, xv, k, 1024, 2048)
            xpart(ge, xvt_d, xv, 4, 1024, 2048)
            xpart(se, xvt_d, xv, 5, 1024, 2048)
            xpart(ge, xvt_d, xv, 6, 1024, 2048)
            xpart(se, xvt_d, xv, 7, 1024, 2048)

            # ---- PE warmup / filler matmuls ----
            # Dependency-free matmuls keep the PE continuously busy through
            # the DMA-bound startup: without them the data-paced gaps reset
            # HAM's busy window and the whole load phase runs at 1.2 GHz.
            def warm(n):
                # rides the S-pair psum ring; in the padded (startup) phase
                # exps drain promptly so the WAW handoff is free
                for _ in range(n):
                    wps = ps_a.tile([128, 2, 512], f32, tag="sa", name="warm_ps")
                    nc.tensor.matmul(
                        wps[:, 0, :], warm_l[:], warm_r[:], start=True, stop=True
                    )
            warm(N_WARMUP)

            # ---- PE filler units -------------------------------------
            def u_qkproj(which, dhc, t4, pad=0):
                # one [128,512] projection unit: q or k, head-dim half dhc,
                # t columns t4*512..+512
                xch, w_sb, b_sb, dst = (
                    (xq, wq_sb, bq_sb, qt_sb) if which == "q" else (xk, wk_sb, bk_sb, kt_sb)
                )
                def emit():
                    ps = ps_b.tile([128, 512], f32, tag="b", name=f"{which}p_ps")
                    for k in range(KC):
                        nc.tensor.matmul(
                            ps[:],
                            w_sb[:, k, dhc * 128 : (dhc + 1) * 128],
                            xch[k][:, t4 * 512 : (t4 + 1) * 512],
                            start=(k == 0),
                            stop=(k == KC - 1),
                        )
                        if pad:
                            warm(pad)
                    nc.scalar.activation(
                        out=dst[dhc][:, t4 * 512 : (t4 + 1) * 512],
                        in_=ps[:],
                        func=Identity,
                        bias=b_sb[:, dhc : dhc + 1],
                        scale=1.0,
                    )
                return emit

            def u_vproj(tb):
                def emit():
                    ps = ps_b.tile([128, 512], f32, tag="b", name="v_ps")
                    for k in range(KC):
                        nc.tensor.matmul(
                            ps[:, 0:DH],
                            xv[k][:, tb * 128 : (tb + 1) * 128],
                            wv_sb[:, k, :],
                            start=(k == 0),
                            stop=(k == KC - 1),
                        )
                    nc.vector.tensor_copy(
                        out=vaug_sb[:, tb, :].rearrange("p (h x) -> p h x", h=HPC)[:, :, 0:HD],
                        in_=ps[:, 0:DH].rearrange("p (h x) -> p h x", h=HPC),
                    )
                return emit

            def u_oproj(m, n2, ob_ref):
                # half output-projection unit: out columns n2*512..+512 of
                # t1 block m. ob_ref[0] holds the bf16 staging tile shared
                # by the two halves of block m.
                def emit():
                    if n2 == 0:
                        ob_ref[0] = outp.tile([128, D], bf16, tag="ob", name="ob")
                    ps = ps_b.tile([128, 512], f32, tag="b", name="op_ps")
                    for dhc in range(2):
                        nc.tensor.matmul(
                            ps[:],
                            ont_sb[dhc][:, m * 128 : (m + 1) * 128],
                            wo_sb[:, dhc, n2 * 512 : (n2 + 1) * 512],
                            start=(dhc == 0),
                            stop=(dhc == 1),
                        )
                    ob = ob_ref[0]
                    nc.vector.tensor_copy(out=ob[:, n2 * 512 : (n2 + 1) * 512], in_=ps[:])
                    if n2 == 1:
                        nc.sync.dma_start(out=out_d[m * 128 : (m + 1) * 128, :], in_=ob[:])
                return emit

            def ops_for(c):
                units = []
                for m in range(4 * c, 4 * c + 4):
                    ob_ref = [None]
                    units.append(u_oproj(m, 0, ob_ref))
                    units.append(u_oproj(m, 1, ob_ref))
                return units

            # ---- attention streams ----------------------------------
            def make_spair(h, c):
                def s_pair(bp):
                    # blocks b0=2bp, b1=2bp+1 share one [128, 2, 512] psum tile
                    hc, hr = h // 2, (h % 2) * 64
                    s_ps = ps_a.tile([128, 2, 512], f32, tag="sa", name="s_ps")
                    pt = ptp.tile([128, 2, 512], bf16, tag="pt", name="pt")
                    geo = []
                    for i in range(2):
                        b = 2 * bp + i
                        r = b - 4 * c
                        off = max(r, 0) * 128
                        w = 512 - off
                        geo.append((b, r, off, w))
                        nc.tensor.matmul(
                            s_ps[:, i, off : off + w],
                            kt_sb[hc][hr : hr + 64, b * 128 : (b + 1) * 128],
                            qt_sb[hc][hr : hr + 64, c * 512 + off : (c + 1) * 512],
                            start=True,
                            stop=True,
                        )
                    if geo[0][1] < 0 and geo[1][1] < 0:
                        # both below the diagonal: one merged exp over 1024 cols
                        nc.scalar.activation(out=pt[:], in_=s_ps[:], func=Exp, scale=0.125)
                    else:
                        for i, (b, r, off, w) in enumerate(geo):
                            nc.scalar.activation(
                                out=pt[:, i, off : off + w],
                                in_=s_ps[:, i, off : off + w],
                                func=Exp,
                                scale=0.125,
                            )
                    for i, (b, r, off, w) in enumerate(geo):
                        if r >= 0:
                            nc.vector.tensor_mul(
                                pt[:, i, off : off + 128],
                                pt[:, i, off : off + 128],
                                tri_sb[:],
                            )
                    return (pt, geo)
                return s_pair

            def pv_block(h, c, b, pairs, o_ps, nblk):
                pt, geo = pairs[b // 2]
                i = b % 2
                _, r, off, w = geo[i]
                nc.tensor.matmul(
                    o_ps[:, off : off + w],
                    vaug_sb[:, b, h * (HD + 1) : (h + 1) * (HD + 1)],
                    pt[:, i, off : off + w],
                    start=(b == 0),
                    stop=(b == nblk - 1),
                )

            def norm(h, c, o_ps):
                # cross-partition hop (psum row 64 -> partition 0) must be a
                # DVE COPY (hw-proven); recip stays partition-aligned after.
                hc, hr = h // 2, (h % 2) * 64
                den = recp.tile([1, 512], f32, tag="den", name="den")
                nc.vector.tensor_copy(out=den[:], in_=o_ps[HD : HD + 1, :])
                rec = recp.tile([1, 512], f32, tag="rec", name="rec")
                nc.vector.reciprocal_approx_fast(out=rec[:], in_=den[:])
                bcb = bcp.tile([64, 512], f32, tag="bcb", name="bcb")
                nc.gpsimd.partition_broadcast(bcb[:], rec[:])
                nc.vector.tensor_mul(
                    ont_sb[hc][hr : hr + 64, c * 512 : (c + 1) * 512],
                    o_ps[0:HD, :],
                    bcb[:],
                )

            # prelude: the two units attention chunk 0 needs, padded so the
            # PE stays busy while the t0 slices stream in
            u_qkproj("q", 0, 0, pad=4)()
            u_qkproj("k", 0, 0, pad=4)()

            # per-chunk drip units, ordered to respect dependencies:
            #   - v(4c..4c+3) precede PV(0, those blocks)  -> stream A slots
            #   - qk(1,0) precede S(2) of chunk 0          -> head-0 round
            #   - qk(*,t) precede chunk t's streams        -> chunk t-1
            #   - op halves of chunk c-1 run in chunk c (norms done)
            dripA = {
                1: [u_vproj(4), u_vproj(5), u_vproj(6), u_vproj(7)],
                2: [u_vproj(8), u_vproj(9), u_vproj(10), u_vproj(11)],
                3: [u_vproj(12), u_vproj(13), u_vproj(14), u_vproj(15)],
            }
            v_c0 = [u_vproj(0), u_vproj(1), u_vproj(2), u_vproj(3)]
            qk_c0_s = [u_qkproj("q", 1, 0), u_qkproj("k", 1, 0)]  # t0 data only
            qk_c0_pv = [u_qkproj("q", 0, 1), u_qkproj("k", 0, 1)] # t1 arrives ~25-28us
            _ops0 = ops_for(0)
            dripH = {
                1: _ops0[:2] + [u_qkproj("q", 1, 1), u_qkproj("k", 1, 1)]
                   + _ops0[2:]
                   + [u_qkproj("q", 0, 2), u_qkproj("k", 0, 2),
                      u_qkproj("q", 1, 2), u_qkproj("k", 1, 2)],
                2: ops_for(1) + [u_qkproj("q", 0, 3), u_qkproj("k", 0, 3),
                                 u_qkproj("q", 1, 3), u_qkproj("k", 1, 3)],
                3: ops_for(2),
            }

            if True:
                # ---- chunk 0: S-first schedule ----
                # All S pairs + t0-based drips run before any PV so nothing
                # waits on x_v; t1 projection units sit in the PV section
                # (their slices arrive ~25-28us, just in time for chunk 1).
                c, nblk, npair = 0, 4, 2
                o_pss = [
                    ps_o.tile([HD + 1, 512], f32, tag=f"ops{h % 2}", name=f"ops{h}")
                    for h in range(HPC)
                ]
                ptss = {}
                for h in range(HPC):
                    sp = make_spair(h, 0)
                    ptss[h] = [sp(0), sp(1)]
                    if h == 0:
                        qk_c0_s[0](); warm(2)
                        qk_c0_s[1](); warm(2)   # q/k dhc1 before S(2)
                for u in (v_c0[0], v_c0[1]):
                    warm(2); u()
                pv_block(0, 0, 0, ptss[0], o_pss[0], nblk)
                pv_block(0, 0, 1, ptss[0], o_pss[0], nblk)
                for u in (v_c0[2], v_c0[3]):
                    warm(1); u()
                pv_block(0, 0, 2, ptss[0], o_pss[0], nblk)
                pv_block(0, 0, 3, ptss[0], o_pss[0], nblk)
                norm(0, 0, o_pss[0])
                for h in range(1, HPC):
                    for b in range(nblk):
                        pv_block(h, 0, b, ptss[h], o_pss[h], nblk)
                    if h - 1 < len(qk_c0_pv):
                        warm(2)
                        qk_c0_pv[h - 1]()
                    norm(h, 0, o_pss[h])
                ptss.clear()

            for c in range(1, NT4):
                nblk = 4 * c + 4
                npair = nblk // 2
                da = list(dripA[c])
                dh = list(dripH[c])
                # interleave so early drips spread over early slots
                na = len(da)

                o_pss = [
                    ps_o.tile([HD + 1, 512], f32, tag=f"ops{h % 2}", name=f"ops{h}")
                    for h in range(HPC)
                ]
                ptss = {}

                # stream A: scores(0) pairs with drips interleaved
                spair0 = make_spair(0, c)
                ptss[0] = []
                for bp in range(npair):
                    ptss[0].append(spair0(bp))
                    take = (na * (bp + 1)) // npair - (na * bp) // npair
                    for _ in range(take):
                        da.pop(0)()
                while da:
                    da.pop(0)()

                # streams B-E: S(h+1) pairs and PV(h) alternate; drips spread
                for h in range(HPC):
                    hn = h + 1
                    spairn = make_spair(hn, c) if hn < HPC else None
                    if hn < HPC:
                        ptss[hn] = []
                    ndrip = len(dh)
                    # budget this head-round's share of the remaining drips
                    share = ndrip // (HPC - h) + (1 if ndrip % (HPC - h) else 0)
                    for bp in range(npair):
                        if spairn is not None:
                            ptss[hn].append(spairn(bp))
                        pv_block(h, c, 2 * bp, ptss[h], o_pss[h], nblk)
                        pv_block(h, c, 2 * bp + 1, ptss[h], o_pss[h], nblk)
                        take = (share * (bp + 1)) // npair - (share * bp) // npair
                        for _ in range(take):
                            if dh:
                                dh.pop(0)()
                    ptss.pop(h)
                    norm(h, c, o_pss[h])
                while dh:
                    dh.pop(0)()

            # keep the PE busy across the last norm's DVE/gpsimd latency
            # so HAM stays un-throttled for the final output projections
            warm(18)

            # final chunk's output projection; warm fillers absorb the
            # CAST-paced bubbles of the 2-buffer psum ring so the tail
            # never re-throttles
            _final = ops_for(3)
            for i, u in enumerate(_final):
                u()
                if i < len(_final) - 1:
                    warm(1)

    nc.compile()
    return nc


def _get_nc():
    if "nc" not in _cache:
        _cache["nc"] = _build()
    return _cache["nc"]


def build_in_maps(query, key, value, Wq, bq, Wk, bk, Wv, bv, Wo, bo):
    query = np.asarray(query, np.float32)
    key = np.asarray(key, np.float32)
    value = np.asarray(value, np.float32)
    Wq_, Wk_, Wv_, Wo_ = (np.asarray(a, np.float32) for a in (Wq, Wk, Wv, Wo))
    bq_, bk_, bv_, bo_ = (np.asarray(a, np.float32) for a in (bq, bk, bv, bo))

    xqt = [np.ascontiguousarray(query[b].T).astype(_BF16) for b in range(B)]
    xkt = [np.ascontiguousarray(key[b].T).astype(_BF16) for b in range(B)]
    xvt = [np.ascontiguousarray(value[b].T).astype(_BF16) for b in range(B)]

    tri = np.tril(np.ones((128, 128), np.float32)).T.astype(_BF16)  # tri[j,i]=1 iff j<=i

    # bv commutes through softmax: fold it into the output bias
    bo_eff = (bo_ + bv_ @ Wo_).astype(np.float32)

    def pack_w(w):  # [KC*128, N] -> [128, KC*N] matching sbuf [p, k, n]
        kc, n = w.shape[0] // 128, w.shape[1]
        return np.ascontiguousarray(
            w.reshape(kc, 128, n).transpose(1, 0, 2).reshape(128, kc * n)
        ).astype(_BF16)

    in_maps = []
    for c in range(N_CORES):
        b, hg = c // 4, c % 4
        sl = slice(hg * DH, (hg + 1) * DH)
        bqk = np.stack(
            [bq_[sl][0:128], bq_[sl][128:256], bk_[sl][0:128], bk_[sl][128:256]],
            axis=1,
        )
        in_maps.append(
            {
                "xqt": xqt[b],
                "xkt": xkt[b],
                "xvt": xvt[b],
                "wq": pack_w(Wq_[:, sl]),
                "wk": pack_w(Wk_[:, sl]),
                "wv": pack_w(Wv_[:, sl]),
                "wo": pack_w(Wo_[sl, :]),
                "bqk": np.ascontiguousarray(bqk, np.float32),
                "tri": tri,
            }
        )

    return in_maps, bo_eff


def kernel(query, key, value, Wq, bq, Wk, bk, Wv, bv, Wo, bo):
    from concourse.bass_utils import run_bass_kernel_spmd

    nc = _get_nc()
    in_maps, bo_eff = build_in_maps(query, key, value, Wq, bq, Wk, bk, Wv, bv, Wo, bo)
    res = run_bass_kernel_spmd(nc, in_maps, list(range(N_CORES)))
    _cache["last_results"] = res

    out = np.empty((B, T, D), np.float32)
    for b in range(B):
        acc = res.results[4 * b]["out"].astype(np.float32)
        for hg in range(1, 4):
            acc = acc + res.results[4 * b + hg]["out"].astype(np.float32)
        out[b] = acc + bo_eff[None, :]
    return out
